# revision 4
# baseline (speedup 1.0000x reference)
"""DeformationGraph Trainium2 kernel.

Strategy (data-parallel over the B*N=65536 query points, 8 cores):
  core c handles batch b=c//2, points [(c%2)*8192, (c%2+1)*8192).
  Each core (identical SPMD program):
    A. Build per-node 4x4 transforms on-chip (MLP + rodrigues + compose),
       keep rows 0..2 as a [J,12] fp16 table in SBUF (tiled [128, 8, 12]).
    B. Per 128-point tile: partial = 2x.n - |n|^2 on the PE as ONE bf16
       matmul with contract dim 4 (lhsT = host-provided [x^T; -1] bf16),
       vector.max (top-8, direct from PSUM), vector.max_index -> indices.
       d2 = |x|^2 - partial is recovered per selected value (|x|^2 is a
       per-point constant so it does not affect the ranking).
    C. Batched skinning weights L = -0.5*ln(clamp(d2, 1e-11, 1)) + 1e-6,
       normalized per point (single Ln pass, no sqrt: avoids activation
       table swaps; the 1e-11 floor kills d2<=0 NaNs; the +1e-6 matches
       the reference's -ln(1-1e-6) behaviour when all 5 dists >= 1).
    D. Per tile: local_scatter builds the sparse weight row [128, J] fp16,
       PE-transpose chunks -> W^T, 8 accumulating matmuls against the
       transform table give A^T[12, 128]; transpose back to A[128, 12].
    E. out = A[:, :3] @ [x, 1] per point (gpsimd), DMA out.
"""

import numpy as np

import concourse.bass as bass
import concourse.mybir as mybir
import concourse.tile as tile
from concourse import bacc
from concourse.masks import make_identity

F32 = mybir.dt.float32
F16 = mybir.dt.float16
BF16 = mybir.dt.bfloat16
U16 = mybir.dt.uint16
I16 = mybir.dt.int16
AF = mybir.ActivationFunctionType
ALU = mybir.AluOpType
AX = mybir.AxisListType

P = 128        # partitions / points per tile
J = 1024       # nodes
NJT = 8        # node tiles
H = 256        # MLP hidden
CND = 69       # cond dim
K = 5          # knn
HALF_PI = 1.5707963267948966

N_CORES = 8
B_FULL, N_FULL = 4, 16384
PTS_PER_CORE = B_FULL * N_FULL // N_CORES  # 8192


def build_kernel(tc, nc, NT, aps):
    (xin, xqin, ndin, condin, rootin, transin, scalein,
     w1in, b1in, w2in, b2in, outd) = aps
    GC = min(16, NT)          # point tiles per weight/output chunk
    NG = NT // GC

    from contextlib import ExitStack
    ctx = ExitStack()
    pers = ctx.enter_context(tc.tile_pool(name="pers", bufs=1))
    work = ctx.enter_context(tc.tile_pool(name="work", bufs=3))
    # PSUM budget (8 banks): psD2 2x2, psWT 2x1, psS 2x1.
    psD2 = ctx.enter_context(tc.tile_pool(name="psD2", bufs=2, space="PSUM"))
    psWT = ctx.enter_context(tc.tile_pool(name="psWT", bufs=2, space="PSUM"))
    psS = ctx.enter_context(tc.tile_pool(name="psS", bufs=2, space="PSUM"))

    # ---------------- constants / inputs ----------------
    ident = pers.tile([P, P], F32)
    make_identity(nc, ident[:])
    ident16 = pers.tile([P, P], F16)
    nc.vector.tensor_copy(ident16[:], ident[:])
    ones1 = pers.tile([1, P], F32)
    nc.vector.memset(ones1[:], 1.0)
    bias_hpi = pers.tile([P, 1], F32)
    nc.vector.memset(bias_hpi[:], HALF_PI)

    x_pt = pers.tile([P, NT, 3], F32)
    nc.sync.dma_start(x_pt[:], xin.ap()[:])
    xq = pers.tile([4, P * NT], BF16)            # [x0;x1;x2;-1] bf16
    nc.sync.dma_start(xq[:], xqin.ap()[:])
    nd = pers.tile([P, NJT, 3], F32)
    nc.sync.dma_start(nd[:], ndin.ap()[:])
    cond_s = pers.tile([CND, 1], F32)
    nc.sync.dma_start(cond_s[:], condin.ap()[:])
    W1s = pers.tile([72, H], F32)
    nc.sync.dma_start(W1s[:], w1in.ap()[:])
    b1s = pers.tile([1, H], F32)
    nc.sync.dma_start(b1s[:], b1in.ap()[:])
    W2sa = pers.tile([P, 6], F32)
    nc.sync.dma_start(W2sa[:], w2in.ap()[0:128, :])
    W2sb = pers.tile([P, 6], F32)
    nc.sync.dma_start(W2sb[:], w2in.ap()[128:256, :])
    b2s = pers.tile([1, 6], F32)
    nc.sync.dma_start(b2s[:], b2in.ap()[:])

    # |x|^2 per point, [128, NT] f32
    xsq3 = pers.tile([P, NT, 3], F32)
    nc.vector.tensor_mul(xsq3[:], x_pt[:], x_pt[:])
    xsq = pers.tile([P, NT], F32)
    nc.vector.tensor_reduce(xsq[:], xsq3[:], axis=AX.X, op=ALU.add)

    # ---------------- node rows [n0,n1,n2,|n|^2] -> rhsE4 [4, J] ---------
    nstg = pers.tile([P, NJT, 4], F32)
    nc.vector.tensor_copy(nstg[:, :, 0:3], nd[:])
    ndsq = pers.tile([P, NJT, 3], F32)
    nc.vector.tensor_mul(ndsq[:], nd[:], nd[:])
    nc.vector.tensor_reduce(nstg[:, :, 3:4], ndsq[:], axis=AX.X, op=ALU.add)

    rhsE4 = pers.tile([4, J], F32)
    for t in range(NJT):
        ps_n4 = psS.tile([4, P], F32, tag="small")
        nc.tensor.transpose(out=ps_n4[:], in_=nstg[:, t, :],
                            identity=ident[:])
        nc.scalar.copy(rhsE4[:, t * P:(t + 1) * P], ps_n4[:])
    # d2 rhs in bf16: rows [2n0, 2n1, 2n2, |n|^2]
    nstg2 = pers.tile([P, NJT, 4], F32)
    nc.vector.tensor_scalar_mul(nstg2[:, :, 0:3], nd[:], 2.0)
    nc.vector.tensor_copy(nstg2[:, :, 3:4], nstg[:, :, 3:4])
    ndEb = pers.tile([4, J], BF16)
    for t in range(NJT):
        ps_n4b = psS.tile([4, P], F32, tag="small")
        nc.tensor.transpose(out=ps_n4b[:], in_=nstg2[:, t, :],
                            identity=ident[:])
        nc.scalar.copy(ndEb[:, t * P:(t + 1) * P], ps_n4b[:])

    cond_bc = pers.tile([CND, P], F32)
    nc.vector.tensor_copy(cond_bc[:], cond_s[:].to_broadcast([CND, P]))
    W1c = pers.tile([CND, H], F32)
    nc.sync.dma_start(W1c[:], w1in.ap()[3:72, :])

    # ---------------- phase A: MLP -> tf6 per node ----------------------
    tf6 = pers.tile([P, 9, 6], F32)     # slot 8 = root orient (partition 0)
    nc.vector.memset(tf6[:, 8, :], 0.0)
    nc.sync.dma_start(tf6[0:1, 8, 0:3], rootin.ap()[:])

    for t in range(NJT):
        hT = [None, None]
        for c in range(2):
            ps_h = psWT.tile([P, P], F32, tag="wt")
            nc.tensor.matmul(ps_h[:], lhsT=W1s[0:3, c * P:(c + 1) * P],
                             rhs=rhsE4[0:3, t * P:(t + 1) * P],
                             start=True, stop=False)
            nc.tensor.matmul(ps_h[:], lhsT=W1c[:, c * P:(c + 1) * P],
                             rhs=cond_bc[:], start=False, stop=False)
            nc.tensor.matmul(ps_h[:], lhsT=b1s[0:1, c * P:(c + 1) * P],
                             rhs=ones1[:], start=False, stop=True)
            h_c = work.tile([P, P], F32, tag="hT")
            hT[c] = h_c
            nc.scalar.activation(hT[c][:], ps_h[:], AF.Relu)
        ps_t6 = psS.tile([6, P], F32, tag="small")
        nc.tensor.matmul(ps_t6[:], lhsT=W2sa[:], rhs=hT[0][:],
                         start=True, stop=False)
        nc.tensor.matmul(ps_t6[:], lhsT=W2sb[:], rhs=hT[1][:],
                         start=False, stop=False)
        nc.tensor.matmul(ps_t6[:], lhsT=b2s[:], rhs=ones1[:],
                         start=False, stop=True)
        t6s = work.tile([6, P], F32, tag="t6s")
        nc.scalar.copy(t6s[:], ps_t6[:])
        ps_tf = psS.tile([P, 6], F32, tag="small")
        nc.tensor.transpose(out=ps_tf[:], in_=t6s[:], identity=ident[0:6, 0:6])
        nc.vector.tensor_copy(tf6[:, t, :], ps_tf[:])

    # ---------------- phase A: batched rodrigues on [128, 9, .] ---------
    _s9n = [0]
    def S9():
        _s9n[0] += 1
        return pers.tile([P, 9], F32, name=f"s9_{_s9n[0]}")
    a = tf6[:, :, 0:3]
    se = pers.tile([P, 9, 3], F32)
    nc.vector.tensor_scalar_add(se[:], a, 1e-8)
    sq = pers.tile([P, 9, 3], F32)
    nc.vector.tensor_mul(sq[:], se[:], se[:])
    ang2 = S9()
    nc.vector.tensor_reduce(ang2[:], sq[:], axis=AX.X, op=ALU.add)
    ang = S9()
    nc.scalar.activation(ang[:], ang2[:], AF.Sqrt)
    cw = S9()
    nc.scalar.activation(cw[:], ang[:], AF.Sin, bias=bias_hpi[:], scale=0.5)
    sh = S9()
    nc.scalar.activation(sh[:], ang[:], AF.Sin, bias=0.0, scale=0.5)
    rai = S9()
    nc.vector.reciprocal(rai[:], ang[:])
    sa = S9()
    nc.vector.tensor_mul(sa[:], sh[:], rai[:])

    qs4 = pers.tile([P, 9, 4], F32)     # unnormalized quat [w, xyz]
    nc.vector.tensor_copy(qs4[:, :, 0:1], cw[:])
    nc.vector.tensor_mul(qs4[:, :, 1:4], a, sa[:].to_broadcast([P, 9, 3]))
    qq = pers.tile([P, 9, 4], F32)
    nc.vector.tensor_mul(qq[:], qs4[:], qs4[:])
    n2 = S9()
    nc.vector.tensor_reduce(n2[:], qq[:], axis=AX.X, op=ALU.add)
    rq = S9()
    nc.vector.reciprocal(rq[:], n2[:])
    qn = pers.tile([P, 9, 4], F32)      # q / |q|^2
    nc.vector.tensor_mul(qn[:], qs4[:], rq[:].to_broadcast([P, 9, 4]))

    def prod(ia, ib):
        o = S9()
        nc.vector.tensor_mul(o[:], qs4[:, :, ia:ia + 1], qn[:, :, ib:ib + 1])
        return o
    w2, x2, y2, z2 = prod(0, 0), prod(1, 1), prod(2, 2), prod(3, 3)
    wx, wy, wz = prod(0, 1), prod(0, 2), prod(0, 3)
    xy, xz, yz = prod(1, 2), prod(1, 3), prod(2, 3)

    D = pers.tile([P, 9, 12], F32)      # [R | t] rows 0..2 flattened
    tmp = S9()

    def diag(col, pa, pb, na, nb):
        nc.vector.tensor_add(tmp[:], pa[:], pb[:])
        nc.vector.tensor_sub(D[:, :, col:col + 1], tmp[:], na[:])
        nc.vector.tensor_sub(D[:, :, col:col + 1], D[:, :, col:col + 1], nb[:])
    diag(0, w2, x2, y2, z2)    # R00
    diag(5, w2, y2, x2, z2)    # R11
    diag(10, w2, z2, x2, y2)   # R22

    def offd(col, pa, pb, sign):
        if sign > 0:
            nc.vector.tensor_add(tmp[:], pa[:], pb[:])
        else:
            nc.vector.tensor_sub(tmp[:], pa[:], pb[:])
        nc.vector.tensor_scalar_mul(D[:, :, col:col + 1], tmp[:], 2.0)
    offd(1, xy, wz, -1)   # R01 = 2(xy - wz)
    offd(2, wy, xz, +1)   # R02 = 2(wy + xz)
    offd(4, wz, xy, +1)   # R10 = 2(wz + xy)
    offd(6, yz, wx, -1)   # R12 = 2(yz - wx)
    offd(8, xz, wy, -1)   # R20 = 2(xz - wy)
    offd(9, wx, yz, +1)   # R21 = 2(wx + yz)
    nc.vector.tensor_copy(D[:, :, 3:12:4], tf6[:, :, 3:6])  # translation col

    # ---------------- phase A: compose with root / scale / trans --------
    bcrow = pers.tile([1, 13], F32)     # [Rr(9) | scale | trans(3)]
    nc.vector.tensor_copy(bcrow[0:1, 0:9],
                          D[0:1, 8, :].rearrange("p (i f) -> p i f", f=4)[:, :, 0:3])
    nc.sync.dma_start(bcrow[0:1, 9:10], scalein.ap()[:])
    nc.sync.dma_start(bcrow[0:1, 10:13], transin.ap()[:])
    ps_bc = psS.tile([P, 13], F32, tag="small")
    nc.tensor.matmul(ps_bc[:], lhsT=ones1[:], rhs=bcrow[:], start=True, stop=True)
    Bc = pers.tile([P, 13], F32)
    nc.scalar.copy(Bc[:], ps_bc[:])

    T12 = pers.tile([P, NJT, 12], F32)
    Dn = D[:, 0:NJT, :].rearrange("p t (i f) -> p t i f", f=4)
    for i in range(3):
        nc.vector.tensor_scalar(T12[:, :, 4 * i:4 * i + 4], Dn[:, :, 0, :],
                                Bc[:, 3 * i:3 * i + 1], None, op0=ALU.mult)
        for jj in (1, 2):
            nc.vector.scalar_tensor_tensor(
                T12[:, :, 4 * i:4 * i + 4], Dn[:, :, jj, :],
                Bc[:, 3 * i + jj:3 * i + jj + 1],
                T12[:, :, 4 * i:4 * i + 4], op0=ALU.mult, op1=ALU.add)
    nc.vector.tensor_scalar(T12[:], T12[:], Bc[:, 9:10], None, op0=ALU.mult)
    st3 = pers.tile([P, 3], F32)
    nc.vector.tensor_mul(st3[:], Bc[:, 10:13], Bc[:, 9:10].to_broadcast([P, 3]))
    for i in range(3):
        nc.vector.tensor_scalar(T12[:, :, 4 * i + 3:4 * i + 4],
                                T12[:, :, 4 * i + 3:4 * i + 4],
                                st3[:, i:i + 1], None, op0=ALU.add)
    Ttab = pers.tile([P, NJT, 12], F16)
    nc.vector.tensor_copy(Ttab[:], T12[:])

    # ---------------- phases B-E over point tiles -----------------------
    V = pers.tile([P, NT, 8], F32)
    Iu = pers.tile([P, NT, 8], U16)
    WN16 = pers.tile([P, NT, 6], F16)
    nc.vector.memset(WN16[:], 0.0)
    A_all = pers.tile([P, NT, 12], F32)

    for g in range(NG):
        tiles = range(g * GC, (g + 1) * GC)
        gs = slice(g * GC, (g + 1) * GC)
        # ---- B: knn matmul + top8 + indices per tile ----
        for t in tiles:
            ps_d2 = psD2.tile([P, J], F32, tag="d2")
            nc.tensor.matmul(ps_d2[:, 0:512],
                             lhsT=xq[:, t * P:(t + 1) * P],
                             rhs=ndEb[:, 0:512], start=True, stop=True)
            nc.tensor.matmul(ps_d2[:, 512:1024],
                             lhsT=xq[:, t * P:(t + 1) * P],
                             rhs=ndEb[:, 512:1024], start=True, stop=True)
            nc.vector.max(out=V[:, t, :], in_=ps_d2[:])
            nc.vector.max_index(out=Iu[:, t, :], in_max=V[:, t, :],
                                in_values=ps_d2[:])
        # ---- C: batched weights for the chunk ----
        nc.gpsimd.memset(Iu[:, gs, 5:6], 65535)  # int16 -1 pad for scatter
        D2 = work.tile([P, GC, 5], F32, tag="wd2")
        # d2 = |x|^2 - partial, clamped to [1e-11, 1]
        nc.vector.tensor_sub(D2[:], xsq[:, gs].to_broadcast([P, GC, 5]),
                             V[:, gs, 0:5])
        nc.vector.tensor_scalar(D2[:], D2[:], 1e-11, 1.0,
                                op0=ALU.max, op1=ALU.min)
        L0 = work.tile([P, GC, 5], F32, tag="wl0")
        nc.scalar.activation(L0[:], D2[:], AF.Ln)
        # L = -0.5*L0 + 1e-6; w = L / sum(L) = (L0 - 2e-6) / (sum(L0) - 1e-5)
        SL = work.tile([P, GC], F32, tag="wsl")
        nc.vector.tensor_reduce(SL[:], L0[:], axis=AX.X, op=ALU.add)
        nc.vector.tensor_scalar_add(SL[:], SL[:], -1e-5)
        RL = work.tile([P, GC], F32, tag="wrl")
        nc.vector.reciprocal(RL[:], SL[:])
        WNf = work.tile([P, GC, 5], F32, tag="wn")
        nc.vector.tensor_scalar_add(WNf[:], L0[:], -2e-6)
        nc.vector.tensor_mul(WN16[:, gs, 0:5], WNf[:],
                             RL[:].to_broadcast([P, GC, 5]))
        # ---- D: scatter + transpose + matmul per tile ----
        for t in tiles:
            Wd = work.tile([P, J], F16, tag="wden")
            nc.gpsimd.local_scatter(
                out_ap=Wd[:], data_ap=WN16[:, t, :],
                idxs_ap=Iu[:, t, 0:6].bitcast(I16),
                channels=P, num_elems=J, num_idxs=6)
            ps_wt = psWT.tile([P, NJT, P], F16, tag="wt")
            for jj in range(NJT):
                nc.tensor.transpose(out=ps_wt[:, jj, :],
                                    in_=Wd[:, jj * P:(jj + 1) * P],
                                    identity=ident16[:])
            WtS = work.tile([P, NJT, P], F16, tag="wts")
            nc.scalar.copy(WtS[:], ps_wt[:])
            if t % 4 == 0:
                ps_at = psS.tile([12, 4, P], F32, tag="small")
            for jj in range(NJT):
                nc.tensor.matmul(ps_at[:, t % 4, :], lhsT=Ttab[:, jj, :],
                                 rhs=WtS[:, jj, :],
                                 start=(jj == 0), stop=(jj == NJT - 1))
            if t % 4 == 3:
                ATS = work.tile([12, 4, P], F32, tag="ats")
                nc.scalar.copy(ATS[:], ps_at[:])
                ps_pa = psS.tile([P, 4, 12], F32, tag="small")
                for i4 in range(4):
                    nc.tensor.transpose(out=ps_pa[:, i4, :], in_=ATS[:, i4, :],
                                        identity=ident[0:12, 0:12])
                g4 = t // 4
                nc.vector.tensor_copy(A_all[:, 4 * g4:4 * g4 + 4, :], ps_pa[:])
        # ---- E: apply transforms, write out (gpsimd) ----
        OUT3 = work.tile([P, GC, 3], F32, tag="out3")
        PRD = work.tile([P, GC, 3], F32, tag="prd")
        SI = work.tile([P, GC], F32, tag="si")
        for i in range(3):
            nc.gpsimd.tensor_mul(PRD[:], A_all[:, gs, 4 * i:4 * i + 3],
                                 x_pt[:, gs, :])
            nc.vector.tensor_reduce(SI[:], PRD[:], axis=AX.X, op=ALU.add)
            nc.gpsimd.tensor_add(OUT3[:, :, i:i + 1], SI[:],
                                 A_all[:, gs, 4 * i + 3:4 * i + 4])
        nc.sync.dma_start(outd.ap()[:, gs, :], OUT3[:])

    ctx.close()


def dram_tensors(nc, NT):
    xin = nc.dram_tensor("xin", [P, NT, 3], F32, kind="ExternalInput")
    xqin = nc.dram_tensor("xqin", [4, P * NT], BF16, kind="ExternalInput")
    ndin = nc.dram_tensor("ndin", [P, NJT, 3], F32, kind="ExternalInput")
    condin = nc.dram_tensor("condin", [CND, 1], F32, kind="ExternalInput")
    rootin = nc.dram_tensor("rootin", [1, 3], F32, kind="ExternalInput")
    transin = nc.dram_tensor("transin", [1, 3], F32, kind="ExternalInput")
    scalein = nc.dram_tensor("scalein", [1, 1], F32, kind="ExternalInput")
    w1in = nc.dram_tensor("w1in", [72, H], F32, kind="ExternalInput")
    b1in = nc.dram_tensor("b1in", [1, H], F32, kind="ExternalInput")
    w2in = nc.dram_tensor("w2in", [H, 6], F32, kind="ExternalInput")
    b2in = nc.dram_tensor("b2in", [1, 6], F32, kind="ExternalInput")
    outd = nc.dram_tensor("outd", [P, NT, 3], F32, kind="ExternalOutput")
    return (xin, xqin, ndin, condin, rootin, transin, scalein,
            w1in, b1in, w2in, b2in, outd)


def build_program(NT=64):
    nc = bacc.Bacc("TRN2", target_bir_lowering=False, debug=False)
    aps = dram_tensors(nc, NT)
    with tile.TileContext(nc) as tc:
        build_kernel(tc, nc, NT, aps)
    nc.compile()
    return nc


def shard_inputs(x, cond_smpl, nodes, smpl_root_orient, smpl_trans, scale,
                 W1, b1, W2, b2, NT=64):
    """Full inputs -> list of 8 per-core input dicts."""
    import ml_dtypes
    npts = P * NT
    xf = np.ascontiguousarray(np.asarray(x, dtype=np.float32)).reshape(-1, 3)
    nodes_t = (np.asarray(nodes, dtype=np.float32)
               .reshape(NJT, P, 3).transpose(1, 0, 2).copy())
    in_maps = []
    for c in range(N_CORES):
        b = (c * npts) // N_FULL
        off = (c * npts) % N_FULL
        xc = xf[b * N_FULL + off: b * N_FULL + off + npts]
        xq = np.empty((4, npts), np.float32)
        xq[0:3] = xc.T
        xq[3] = -1.0
        xc = xc.reshape(NT, P, 3).transpose(1, 0, 2).copy()
        in_maps.append({
            "xin": xc,
            "xqin": xq.astype(ml_dtypes.bfloat16),
            "ndin": nodes_t,
            "condin": np.asarray(cond_smpl[b], np.float32).reshape(CND, 1),
            "rootin": np.asarray(smpl_root_orient[b], np.float32).reshape(1, 3),
            "transin": np.asarray(smpl_trans[b], np.float32).reshape(1, 3),
            "scalein": np.asarray(scale[b], np.float32).reshape(1, 1),
            "w1in": np.asarray(W1, np.float32),
            "b1in": np.asarray(b1, np.float32).reshape(1, H),
            "w2in": np.asarray(W2, np.float32),
            "b2in": np.asarray(b2, np.float32).reshape(1, 6),
        })
    return in_maps


def unshard_output(results, NT=64):
    outs = []
    for c in range(N_CORES):
        oc = results[c]["outd"]  # [P, NT, 3]
        outs.append(oc.transpose(1, 0, 2).reshape(P * NT, 3))
    full = np.concatenate(outs, axis=0)
    return full.reshape(B_FULL, N_FULL, 3).astype(np.float32)


_prog_cache = {}


def kernel(**inputs):
    from concourse.bass_utils import run_bass_kernel_spmd
    NT = 64
    if NT not in _prog_cache:
        _prog_cache[NT] = build_program(NT)
    nc = _prog_cache[NT]
    in_maps = shard_inputs(**inputs, NT=NT)
    res = run_bass_kernel_spmd(nc, in_maps, core_ids=list(range(N_CORES)))
    return unshard_output(res.results, NT=NT)
